# revision 9
# baseline (speedup 1.0000x reference)
"""DeltaSynapse kernel for Trainium2 (8 NeuronCores, SPMD).

Reference computation:
    Xpre[b,e,o] = sum_d delaymap[d,e,o] * Xd[d,b,e]
    I[b,o]      = sum_e (signs*W)[e,o] * Xpre[b,e,o]

Folded:  I[b,o] = sum_{d,e} (delaymap[d,e,o] * Weff[e,o]) * Xd[d,b,e]
i.e. a sum of D matmuls  I += Xd[d] @ (delaymap[d] . Weff).

Sharding: shard the contraction (pre-neuron e) dim across the 8 cores
(256 rows each). Each core reads its own e-slice of delaymap/W/signs/Xd
and produces a full [16, 2048] partial output; the host sums the 8
partials.

v4: fp16 host shards, all input DMAs HWDGE on the sync queue. Critical
chain = DVE tensor_tensor stream (dm*weff, ~19 us): the head is
shortened by splitting W/signs per e-chunk so both weff multiplies
complete while slab 0 is still in flight, and every TT reads a flat
[P, D, w] region (v3's d-sliced APs ran ~40% slower). Tail ranges
(128/128) live in one DMA slab but as separately-stacked flat blocks.
PSUM copies + output DMAs on the scalar engine; gpsimd idle (its
tensor ops measured ~6x slower than modeled).
"""

import numpy as np

D, B, N = 8, 16, 2048
NCORES = 8
P = 128                 # SBUF partitions / matmul contraction tile
ESH = N // NCORES       # per-core pre-dim shard = 256
ECH = ESH // P          # e-chunks per core = 2

# DMA slabs: (name, [list of o-ranges]); each slab tile is
# [P, NR, ECH, D, w] with equal-width ranges stacked flat.
DMA_SLABS = [
    ("dm0", [(0, 512)]),
    ("dm1", [(512, 1024)]),
    ("dm2", [(1024, 1536)]),
    ("dm3", [(1536, 1792)]),
    ("dm4", [(1792, 1920), (1920, 2048)]),
]

_prog_cache = {}


def _build_program():
    from concourse import bacc, tile
    from concourse import mybir

    f32 = mybir.dt.float32
    f16 = mybir.dt.float16

    nc = bacc.Bacc()
    # Host-prepared layouts (see kernel() below), all fp16 in HBM:
    #   dm{i}: [P, NR, ECH, D, w]  delaymap o-slab (flat per range/chunk)
    #   wsa/wsb: [P, 2, N]         (W, signs) rows, e-chunk a/b
    #   xd   : [P, ECH, D, B]      Xd slice transposed
    f8 = mybir.dt.float8e4
    dram = {}
    for name, ranges in DMA_SLABS:
        w = ranges[0][1] - ranges[0][0]
        dram[name] = nc.dram_tensor(
            name, [P, len(ranges), ECH, D, w], f16, kind="ExternalInput"
        )
    wt = nc.dram_tensor("wt", [P, ECH, N], f16, kind="ExternalInput")
    sg = nc.dram_tensor("sg", [P, ECH, N], f8, kind="ExternalInput")
    xd = nc.dram_tensor("xd", [P, ECH, D, B], f16, kind="ExternalInput")
    out = nc.dram_tensor("out", [B, N], f32, kind="ExternalOutput")

    with tile.TileContext(nc) as tc:
        with (
            tc.tile_pool(name="const", bufs=1) as cpool,
            tc.tile_pool(name="dm", bufs=3) as dmpool,
            tc.tile_pool(name="wd", bufs=6) as wdpool,
            tc.tile_pool(name="psum", bufs=1, space="PSUM") as ppool,
            tc.tile_pool(name="outp", bufs=6) as opool,
        ):
            wt_t = cpool.tile([P, ECH, N], f16)
            sg_t = cpool.tile([P, ECH, N], f8)
            weff = cpool.tile([P, ECH, N], f16)
            xd_t = cpool.tile([P, ECH, D, B], f16)

            dm_tiles = {}
            for name, ranges in DMA_SLABS:
                w = ranges[0][1] - ranges[0][0]
                dm_tiles[name] = dmpool.tile(
                    [P, len(ranges), ECH, D, w], f16, tag="dmslab", name=name
                )

            # All input DMAs HWDGE on the sync queue, dependency order:
            # W/signs land before slab 0 so weff never stalls DVE.
            nc.sync.dma_start(wt_t[:], wt[:])
            nc.sync.dma_start(sg_t[:], sg[:])
            nc.sync.dma_start(xd_t[:], xd[:])
            for name, _ in DMA_SLABS:
                nc.sync.dma_start(dm_tiles[name][:], dram[name][:])

            # Weff = W * signs per e-chunk (DVE, head of the chain; the
            # fp8 operand drops this to 1x mode but it hides under the
            # first delaymap slab's DMA).
            nc.vector.tensor_mul(weff[:, 0, :], wt_t[:, 0], sg_t[:, 0])
            nc.vector.tensor_mul(weff[:, 1, :], wt_t[:, 1], sg_t[:, 1])

            psum = ppool.tile([B, N], f32)
            for name, ranges in DMA_SLABS:
                dm_t = dm_tiles[name]
                for r, (o0, o1) in enumerate(ranges):
                    olen = o1 - o0
                    wd_ts = []
                    for c in range(ECH):
                        wd_t = wdpool.tile([P, D, olen], f16, tag="wd")
                        nc.vector.tensor_mul(
                            wd_t[:],
                            dm_t[:, r, c],
                            weff[:, c, o0:o1].unsqueeze(1).broadcast_to(
                                [P, D, olen]
                            ),
                        )
                        wd_ts.append(wd_t)
                    for c in range(ECH):
                        for d in range(D):
                            nc.tensor.matmul(
                                psum[:, o0:o1],
                                xd_t[:, c, d, :],
                                wd_ts[c][:, d, :],
                                start=(c == 0 and d == 0),
                                stop=(c == ECH - 1 and d == D - 1),
                            )
                    out_t = opool.tile([B, olen], f32, tag="out",
                                       name=f"o{o0}")
                    nc.scalar.copy(out_t[:], psum[:, o0:o1])
                    nc.scalar.dma_start(out[:, o0:o1], out_t[:])

    nc.compile()
    return nc


def _get_program():
    if "nc" not in _prog_cache:
        _prog_cache["nc"] = _build_program()
    return _prog_cache["nc"]


def _shard_inputs(Xd, delaymap, W, signs):
    """Layout permutation/slicing + fp16 cast -> per-core input maps."""
    Xd = np.asarray(Xd, dtype=np.float32)
    delaymap = np.asarray(delaymap, dtype=np.float32)
    W = np.asarray(W, dtype=np.float32)
    signs = np.asarray(signs, dtype=np.float32)

    in_maps = []
    for k in range(NCORES):
        esl = slice(k * ESH, (k + 1) * ESH)
        # delaymap [D, ESH, N] -> [P, ECH, D, N] fp16
        dm_pcd = (
            delaymap[:, esl, :]
            .reshape(D, ECH, P, N)
            .transpose(2, 1, 0, 3)
            .astype(np.float16)
        )
        m = {}
        for name, ranges in DMA_SLABS:
            # [P, NR, ECH, D, w]
            m[name] = np.ascontiguousarray(
                np.stack([dm_pcd[:, :, :, o0:o1] for o0, o1 in ranges],
                         axis=1)
            )
        # W rows fp16 [P, ECH, N]; signs rows fp8 (exact for -1/0/+1)
        import ml_dtypes
        m["wt"] = np.ascontiguousarray(
            W[esl].reshape(ECH, P, N).transpose(1, 0, 2).astype(np.float16)
        )
        m["sg"] = np.ascontiguousarray(
            signs[esl].reshape(ECH, P, N).transpose(1, 0, 2)
        ).astype(ml_dtypes.float8_e4m3)
        # Xd [D, B, ESH] -> [P, ECH, D, B] fp16
        m["xd"] = np.ascontiguousarray(
            Xd[:, :, esl].reshape(D, B, ECH, P).transpose(3, 2, 0, 1)
        ).astype(np.float16)
        in_maps.append(m)
    return in_maps


def _run(in_maps, trace=False, **kw):
    from concourse.bass_utils import run_bass_kernel_spmd

    nc = _get_program()
    return run_bass_kernel_spmd(nc, in_maps, list(range(NCORES)), trace=trace, **kw)


def _gather(res):
    acc = np.zeros((B, N), dtype=np.float64)
    for k in range(NCORES):
        acc += res.results[k]["out"].astype(np.float64)
    return acc.astype(np.float32)


def kernel(Xd, X, delaymap, W, signs):
    in_maps = _shard_inputs(Xd, delaymap, W, signs)
    return _gather(_run(in_maps))


# revision 11
# speedup vs baseline: 1.0803x; 1.0803x over previous
"""DeltaSynapse kernel for Trainium2 (8 NeuronCores, SPMD).

Reference computation:
    Xpre[b,e,o] = sum_d delaymap[d,e,o] * Xd[d,b,e]
    I[b,o]      = sum_e (signs*W)[e,o] * Xpre[b,e,o]

Folded:  I[b,o] = sum_{d,e} (delaymap[d,e,o] * Weff[e,o]) * Xd[d,b,e]
i.e. a sum of D matmuls  I += Xd[d] @ (delaymap[d] . Weff).

Sharding: shard the contraction (pre-neuron e) dim across the 8 cores
(256 rows each). Each core reads its own e-slice of delaymap/W/signs/Xd
and produces a full [16, 2048] partial output; the host sums the 8
partials.

v6: fp16 host shards, all input DMAs HWDGE on the sync queue in
dependency order (W/signs -> Xd -> delaymap o-major). Every delaymap
slab has its own resident SBUF buffer so no DMA issue ever waits on a
buffer-recycle semaphore. The PE is pre-warmed with dummy matmuls
during the otherwise-idle head so the HAM clock gate opens before the
real matmul stream starts. The last o-range accumulates its two
e-chunks into two separate PSUM banks (independent accumulation
groups -> overlapped matmul access latency) and a DVE add fuses them
during the PSUM->SBUF copy. o-ranges taper so the post-stream tail is
short.
"""

import numpy as np

D, B, N = 8, 16, 2048
NCORES = 8
P = 128                 # SBUF partitions / matmul contraction tile
ESH = N // NCORES       # per-core pre-dim shard = 256
ECH = ESH // P          # e-chunks per core = 2

# DMA slabs: (name, [list of o-ranges]); each slab tile is
# [P, NR, ECH, D, w] with equal-width ranges stacked flat.
DMA_SLABS = [
    ("dm0", [(0, 512)]),
    ("dm1", [(512, 1024)]),
    ("dm2", [(1024, 1536)]),
    ("dm3", [(1536, 1792)]),
    ("dm4", [(1792, 1920), (1920, 2048)]),
]
LAST = (1920, 2048)     # dual-PSUM range
NWARM = 8               # dummy matmuls to open the PE clock gate


def _build_program():
    from concourse import bacc, tile
    from concourse import mybir

    f32 = mybir.dt.float32
    f16 = mybir.dt.float16

    nc = bacc.Bacc()
    # Host-prepared layouts (see kernel() below), all fp16 in HBM:
    #   dm{i}: [P, NR, ECH, D, w]  delaymap o-slab (flat per range/chunk)
    #   wsa/wsb: [P, 2, N]         (W, signs) rows, e-chunk a/b
    #   xd   : [P, ECH, D, B]      Xd slice transposed
    dram = {}
    for name, ranges in DMA_SLABS:
        w = ranges[0][1] - ranges[0][0]
        dram[name] = nc.dram_tensor(
            name, [P, len(ranges), ECH, D, w], f16, kind="ExternalInput"
        )
    wsa = nc.dram_tensor("wsa", [P, 2, N], f16, kind="ExternalInput")
    wsb = nc.dram_tensor("wsb", [P, 2, N], f16, kind="ExternalInput")
    xd = nc.dram_tensor("xd", [P, ECH, D, B], f16, kind="ExternalInput")
    out = nc.dram_tensor("out", [B, N], f32, kind="ExternalOutput")

    with tile.TileContext(nc) as tc:
        with (
            tc.tile_pool(name="const", bufs=1) as cpool,
            tc.tile_pool(name="dm", bufs=5) as dmpool,
            tc.tile_pool(name="wd", bufs=6) as wdpool,
            tc.tile_pool(name="psum", bufs=1, space="PSUM") as ppool,
            tc.tile_pool(name="outp", bufs=6) as opool,
        ):
            wsa_t = cpool.tile([P, 2, N], f16)
            wsb_t = cpool.tile([P, 2, N], f16)
            weff = cpool.tile([P, ECH, N], f16)
            xd_t = cpool.tile([P, ECH, D, B], f16)

            dm_tiles = {}
            for name, ranges in DMA_SLABS:
                w = ranges[0][1] - ranges[0][0]
                dm_tiles[name] = dmpool.tile(
                    [P, len(ranges), ECH, D, w], f16, tag="dmslab", name=name
                )

            # All input DMAs HWDGE on the sync queue, dependency order:
            # both ws chunks land before slab 0 so weff never stalls DVE.
            nc.sync.dma_start(wsa_t[:], wsa[:])
            nc.sync.dma_start(wsb_t[:], wsb[:])
            nc.sync.dma_start(xd_t[:], xd[:])
            for name, _ in DMA_SLABS:
                nc.sync.dma_start(dm_tiles[name][:], dram[name][:])

            # Weff = W * signs per e-chunk (DVE, head of the chain).
            nc.vector.tensor_mul(weff[:, 0, :], wsa_t[:, 0], wsa_t[:, 1])
            nc.vector.tensor_mul(weff[:, 1, :], wsb_t[:, 0], wsb_t[:, 1])

            # PE pre-warm: dummy matmuls on already-landed tiles keep the
            # PE busy ~3.4us so the HAM clock gate is open (2.4 GHz) by
            # the time the first real matmul issues. Results are dumped
            # into a scratch PSUM bank and never read.
            pwarm = ppool.tile([B, 512], f32, tag="pwarm")
            for _ in range(NWARM):
                nc.tensor.matmul(
                    pwarm[:], xd_t[:, 0, 0, :], wsa_t[:, 0, 0:512],
                    start=True, stop=True,
                )

            psum = ppool.tile([B, N], f32)
            psum2 = ppool.tile([B, LAST[1] - LAST[0]], f32)
            for name, ranges in DMA_SLABS:
                dm_t = dm_tiles[name]
                for r, (o0, o1) in enumerate(ranges):
                    olen = o1 - o0
                    last = (o0, o1) == LAST
                    wd_ts = []
                    for c in range(ECH):
                        wd_t = wdpool.tile([P, D, olen], f16, tag="wd")
                        nc.vector.tensor_mul(
                            wd_t[:],
                            dm_t[:, r, c],
                            weff[:, c, o0:o1].unsqueeze(1).broadcast_to(
                                [P, D, olen]
                            ),
                        )
                        wd_ts.append(wd_t)
                    if last:
                        # two independent accumulation groups (separate
                        # PSUM banks) -> back-to-back matmuls overlap
                        for d in range(D):
                            for c, pt in ((0, psum[:, o0:o1]), (1, psum2[:])):
                                nc.tensor.matmul(
                                    pt, xd_t[:, c, d, :], wd_ts[c][:, d, :],
                                    start=(d == 0), stop=(d == D - 1),
                                )
                    else:
                        for c in range(ECH):
                            for d in range(D):
                                nc.tensor.matmul(
                                    psum[:, o0:o1],
                                    xd_t[:, c, d, :],
                                    wd_ts[c][:, d, :],
                                    start=(c == 0 and d == 0),
                                    stop=(c == ECH - 1 and d == D - 1),
                                )
                    out_t = opool.tile([B, olen], f32, tag="out",
                                       name=f"o{o0}")
                    if last:
                        # DVE may read only one PSUM operand per instr
                        nc.scalar.copy(out_t[:], psum[:, o0:o1])
                        nc.vector.tensor_add(out_t[:], out_t[:], psum2[:])
                    else:
                        nc.scalar.copy(out_t[:], psum[:, o0:o1])
                    nc.scalar.dma_start(out[:, o0:o1], out_t[:])

    nc.compile()
    return nc


_prog_cache = {}


def _get_program():
    if "nc" not in _prog_cache:
        _prog_cache["nc"] = _build_program()
    return _prog_cache["nc"]


def _shard_inputs(Xd, delaymap, W, signs):
    """Layout permutation/slicing + fp16 cast -> per-core input maps."""
    Xd = np.asarray(Xd, dtype=np.float32)
    delaymap = np.asarray(delaymap, dtype=np.float32)
    W = np.asarray(W, dtype=np.float32)
    signs = np.asarray(signs, dtype=np.float32)

    in_maps = []
    for k in range(NCORES):
        esl = slice(k * ESH, (k + 1) * ESH)
        # delaymap [D, ESH, N] -> [P, ECH, D, N] fp16
        dm_pcd = (
            delaymap[:, esl, :]
            .reshape(D, ECH, P, N)
            .transpose(2, 1, 0, 3)
            .astype(np.float16)
        )
        m = {}
        for name, ranges in DMA_SLABS:
            # [P, NR, ECH, D, w]
            m[name] = np.ascontiguousarray(
                np.stack([dm_pcd[:, :, :, o0:o1] for o0, o1 in ranges],
                         axis=1)
            )
        # W/signs rows per e-chunk -> [P, 2, N] fp16 each
        wk = W[esl].reshape(ECH, P, N).astype(np.float16)
        sk = signs[esl].reshape(ECH, P, N).astype(np.float16)
        m["wsa"] = np.ascontiguousarray(np.stack([wk[0], sk[0]], axis=1))
        m["wsb"] = np.ascontiguousarray(np.stack([wk[1], sk[1]], axis=1))
        # Xd [D, B, ESH] -> [P, ECH, D, B] fp16
        m["xd"] = np.ascontiguousarray(
            Xd[:, :, esl].reshape(D, B, ECH, P).transpose(3, 2, 0, 1)
        ).astype(np.float16)
        in_maps.append(m)
    return in_maps


def _run(in_maps, trace=False, **kw):
    from concourse.bass_utils import run_bass_kernel_spmd

    nc = _get_program()
    return run_bass_kernel_spmd(nc, in_maps, list(range(NCORES)), trace=trace, **kw)


def _gather(res):
    acc = np.zeros((B, N), dtype=np.float64)
    for k in range(NCORES):
        acc += res.results[k]["out"].astype(np.float64)
    return acc.astype(np.float32)


def kernel(Xd, X, delaymap, W, signs):
    in_maps = _shard_inputs(Xd, delaymap, W, signs)
    return _gather(_run(in_maps))


# revision 13
# speedup vs baseline: 1.1413x; 1.0564x over previous
"""DeltaSynapse kernel for Trainium2 (8 NeuronCores, SPMD).

Reference computation:
    Xpre[b,e,o] = sum_d delaymap[d,e,o] * Xd[d,b,e]
    I[b,o]      = sum_e (signs*W)[e,o] * Xpre[b,e,o]

Folded:  I[b,o] = sum_{d,e} (delaymap[d,e,o] * Weff[e,o]) * Xd[d,b,e]
i.e. a sum of D matmuls  I += Xd[d] @ (delaymap[d] . Weff).

Sharding: shard the contraction (pre-neuron e) dim across the 8 cores
(256 rows each). Each core reads its own e-slice of delaymap/W/signs/Xd
and produces a full [16, 2048] partial output; the host sums the 8
partials.

v6: fp16 host shards, all input DMAs HWDGE on the sync queue in
dependency order (W/signs -> Xd -> delaymap o-major). Every delaymap
slab has its own resident SBUF buffer so no DMA issue ever waits on a
buffer-recycle semaphore. The PE is pre-warmed with dummy matmuls
during the otherwise-idle head so the HAM clock gate opens before the
real matmul stream starts. The last o-range accumulates its two
e-chunks into two separate PSUM banks (independent accumulation
groups -> overlapped matmul access latency) and a DVE add fuses them
during the PSUM->SBUF copy. o-ranges taper so the post-stream tail is
short.
"""

import numpy as np

D, B, N = 8, 16, 2048
NCORES = 8
P = 128                 # SBUF partitions / matmul contraction tile
ESH = N // NCORES       # per-core pre-dim shard = 256
ECH = ESH // P          # e-chunks per core = 2

# DMA slabs: (name, [list of o-ranges]); each slab tile is
# [P, NR, ECH, D, w] with equal-width ranges stacked flat.
DMA_SLABS = [
    ("dm0", [(0, 512)]),
    ("dm1", [(512, 1024)]),
    ("dm2", [(1024, 1536)]),
    ("dm3", [(1536, 1792)]),
    ("dm4", [(1792, 1920), (1920, 2048)]),
]
LAST = (1920, 2048)     # dual-PSUM range
NWARM = 12              # dummy matmuls to open the PE clock gate


def _build_program():
    from concourse import bacc, tile
    from concourse import mybir

    f32 = mybir.dt.float32
    f16 = mybir.dt.float16

    nc = bacc.Bacc()
    # Host-prepared layouts (see kernel() below), all fp16 in HBM:
    #   dm{i}: [P, NR, ECH, D, w]  delaymap o-slab (flat per range/chunk)
    #   wsa/wsb: [P, 2, N]         (W, signs) rows, e-chunk a/b
    #   xd   : [P, ECH, D, B]      Xd slice transposed
    dram = {}
    for name, ranges in DMA_SLABS:
        w = ranges[0][1] - ranges[0][0]
        dram[name] = nc.dram_tensor(
            name, [P, len(ranges), ECH, D, w], f16, kind="ExternalInput"
        )
    wsa = nc.dram_tensor("wsa", [P, 2, N], f16, kind="ExternalInput")
    wsb = nc.dram_tensor("wsb", [P, 2, N], f16, kind="ExternalInput")
    xd = nc.dram_tensor("xd", [P, ECH, D, B], f16, kind="ExternalInput")
    out = nc.dram_tensor("out", [B, N], f32, kind="ExternalOutput")

    with tile.TileContext(nc) as tc:
        with (
            tc.tile_pool(name="const", bufs=1) as cpool,
            tc.tile_pool(name="dm", bufs=5) as dmpool,
            tc.tile_pool(name="wd", bufs=6) as wdpool,
            tc.tile_pool(name="psum", bufs=1, space="PSUM") as ppool,
            tc.tile_pool(name="outp", bufs=6) as opool,
        ):
            wsa_t = cpool.tile([P, 2, N], f16)
            wsb_t = cpool.tile([P, 2, N], f16)
            weff = cpool.tile([P, ECH, N], f16)
            xd_t = cpool.tile([P, ECH, D, B], f16)

            dm_tiles = {}
            for name, ranges in DMA_SLABS:
                w = ranges[0][1] - ranges[0][0]
                dm_tiles[name] = dmpool.tile(
                    [P, len(ranges), ECH, D, w], f16, tag="dmslab", name=name
                )

            # All input DMAs HWDGE on the sync queue, dependency order:
            # both ws chunks land before slab 0 so weff never stalls DVE.
            nc.sync.dma_start(wsa_t[:], wsa[:])
            nc.sync.dma_start(wsb_t[:], wsb[:])
            nc.sync.dma_start(xd_t[:], xd[:])
            # one DMA per (slab, e-chunk): halves the descriptor set per
            # completion sem (a single straggler SDMA engine delays the
            # consumer TT less) and lets each chunk's TT start earlier
            for name, _ in DMA_SLABS:
                for c in range(ECH):
                    nc.sync.dma_start(dm_tiles[name][:, :, c],
                                      dram[name][:, :, c])

            # Weff = W * signs per e-chunk (DVE, head of the chain).
            nc.vector.tensor_mul(weff[:, 0, :], wsa_t[:, 0], wsa_t[:, 1])
            nc.vector.tensor_mul(weff[:, 1, :], wsb_t[:, 0], wsb_t[:, 1])

            # PE pre-warm: dummy matmuls on already-landed tiles keep the
            # PE busy ~3.4us so the HAM clock gate is open (2.4 GHz) by
            # the time the first real matmul issues. Results are dumped
            # into a scratch PSUM bank and never read.
            pwarm = ppool.tile([B, 512], f32, tag="pwarm")
            for _ in range(NWARM):
                nc.tensor.matmul(
                    pwarm[:], xd_t[:, 0, 0, :], wsa_t[:, 0, 0:512],
                    start=True, stop=True,
                )

            psum = ppool.tile([B, N], f32)
            psum2 = ppool.tile([B, LAST[1] - LAST[0]], f32)
            for name, ranges in DMA_SLABS:
                dm_t = dm_tiles[name]
                for r, (o0, o1) in enumerate(ranges):
                    olen = o1 - o0
                    last = (o0, o1) == LAST
                    wd_ts = []
                    for c in range(ECH):
                        wd_t = wdpool.tile([P, D, olen], f16, tag="wd")
                        nc.vector.tensor_mul(
                            wd_t[:],
                            dm_t[:, r, c],
                            weff[:, c, o0:o1].unsqueeze(1).broadcast_to(
                                [P, D, olen]
                            ),
                        )
                        wd_ts.append(wd_t)
                    if last:
                        # two independent accumulation groups (separate
                        # PSUM banks) -> back-to-back matmuls overlap
                        for d in range(D):
                            for c, pt in ((0, psum[:, o0:o1]), (1, psum2[:])):
                                nc.tensor.matmul(
                                    pt, xd_t[:, c, d, :], wd_ts[c][:, d, :],
                                    start=(d == 0), stop=(d == D - 1),
                                )
                    else:
                        for c in range(ECH):
                            for d in range(D):
                                nc.tensor.matmul(
                                    psum[:, o0:o1],
                                    xd_t[:, c, d, :],
                                    wd_ts[c][:, d, :],
                                    start=(c == 0 and d == 0),
                                    stop=(c == ECH - 1 and d == D - 1),
                                )
                    out_t = opool.tile([B, olen], f32, tag="out",
                                       name=f"o{o0}")
                    if last:
                        # DVE may read only one PSUM operand per instr
                        nc.scalar.copy(out_t[:], psum[:, o0:o1])
                        nc.vector.tensor_add(out_t[:], out_t[:], psum2[:])
                    else:
                        nc.scalar.copy(out_t[:], psum[:, o0:o1])
                    nc.scalar.dma_start(out[:, o0:o1], out_t[:])

    nc.compile()
    return nc


_prog_cache = {}


def _get_program():
    if "nc" not in _prog_cache:
        _prog_cache["nc"] = _build_program()
    return _prog_cache["nc"]


def _shard_inputs(Xd, delaymap, W, signs):
    """Layout permutation/slicing + fp16 cast -> per-core input maps."""
    Xd = np.asarray(Xd, dtype=np.float32)
    delaymap = np.asarray(delaymap, dtype=np.float32)
    W = np.asarray(W, dtype=np.float32)
    signs = np.asarray(signs, dtype=np.float32)

    in_maps = []
    for k in range(NCORES):
        esl = slice(k * ESH, (k + 1) * ESH)
        # delaymap [D, ESH, N] -> [P, ECH, D, N] fp16
        dm_pcd = (
            delaymap[:, esl, :]
            .reshape(D, ECH, P, N)
            .transpose(2, 1, 0, 3)
            .astype(np.float16)
        )
        m = {}
        for name, ranges in DMA_SLABS:
            # [P, NR, ECH, D, w]
            m[name] = np.ascontiguousarray(
                np.stack([dm_pcd[:, :, :, o0:o1] for o0, o1 in ranges],
                         axis=1)
            )
        # W/signs rows per e-chunk -> [P, 2, N] fp16 each
        wk = W[esl].reshape(ECH, P, N).astype(np.float16)
        sk = signs[esl].reshape(ECH, P, N).astype(np.float16)
        m["wsa"] = np.ascontiguousarray(np.stack([wk[0], sk[0]], axis=1))
        m["wsb"] = np.ascontiguousarray(np.stack([wk[1], sk[1]], axis=1))
        # Xd [D, B, ESH] -> [P, ECH, D, B] fp16
        m["xd"] = np.ascontiguousarray(
            Xd[:, :, esl].reshape(D, B, ECH, P).transpose(3, 2, 0, 1)
        ).astype(np.float16)
        in_maps.append(m)
    return in_maps


def _run(in_maps, trace=False, **kw):
    from concourse.bass_utils import run_bass_kernel_spmd

    nc = _get_program()
    return run_bass_kernel_spmd(nc, in_maps, list(range(NCORES)), trace=trace, **kw)


def _gather(res):
    acc = np.zeros((B, N), dtype=np.float64)
    for k in range(NCORES):
        acc += res.results[k]["out"].astype(np.float64)
    return acc.astype(np.float32)


def kernel(Xd, X, delaymap, W, signs):
    in_maps = _shard_inputs(Xd, delaymap, W, signs)
    return _gather(_run(in_maps))
